# revision 3
# baseline (speedup 1.0000x reference)
# Koopman operator model on 8 NeuronCores — pure data parallel over batch.
#
# Device kernel works in a feature-major, token-column layout: every tensor is
# [features, tokens-columns]. The 32768 tokens per core are split into 4
# quarters of 8192; column c carries tokens {q*8192+c}. States are packed
# [128, 8192] (4 quarters x 32 features) so every DMA uses all 128 SBUF
# partitions. The MLP runs as two "pair" streams (quarters 0+1 on partitions
# 0-63 / 64-127 of the weights via block-diagonal packing, quarters 2+3 via
# tile_position row/col offsets), and the prediction head accumulates three
# matmuls (actions quad-block-diag + two pair h2@W3AC) into one PSUM tile.
#
# Math folding (host, exact): normalization into W1/b1; the head
#   pred = lat@A@C + norm_act@B@C  ==  h2@(W3@A@C) + act@(B@C/ascale) + cbias
# so the kernel is 4 matmul passes over tokens + the small action pass.
# All matmul operands are fp16 (1 cycle/row on PE vs 4 for fp32); accumulation
# is fp32 in PSUM; bias+ReLU epilogues run on ACT/DVE; outputs stored fp32.
# Measured end-to-end scale-relative error ~5e-4.

import numpy as np

_N_CORES = 8
_B, _S = 256, 1024
_SD, _AD, _LD = 32, 8, 64
_BPC = _B // _N_CORES            # batches per core
_TOK = _BPC * _S                 # tokens per core (32768)
_Q = _TOK // 4                   # columns per core (4 stacked quarters)
_NT = 512                        # matmul moving-dim tile (PSUM bank limit)
_SUPER = 1024                    # columns per DMA super-tile
_NSUP = _Q // _SUPER
_NJ = _SUPER // _NT

_prog_cache = {}


def _build_program():
    if "nc" in _prog_cache:
        return _prog_cache["nc"]

    import concourse.mybir as mybir
    import concourse.tile as tile
    from concourse import bacc

    f16, f32 = mybir.dt.float16, mybir.dt.float32
    Relu = mybir.ActivationFunctionType.Relu
    Ident = mybir.ActivationFunctionType.Identity
    add, mx = mybir.AluOpType.add, mybir.AluOpType.max

    nc = bacc.Bacc("TRN2", target_bir_lowering=False, debug=False,
                   num_devices=_N_CORES)

    X = nc.dram_tensor("X", [128, _Q], f16, kind="ExternalInput")
    Aa = nc.dram_tensor("Aa", [32, _Q], f16, kind="ExternalInput")
    W1b = nc.dram_tensor("W1b", [128, 128], f16, kind="ExternalInput")
    W2b = nc.dram_tensor("W2b", [128, 128], f16, kind="ExternalInput")
    W3b = nc.dram_tensor("W3b", [128, 128], f16, kind="ExternalInput")
    WPb = nc.dram_tensor("WPb", [128, 64], f16, kind="ExternalInput")
    WBb = nc.dram_tensor("WBb", [32, 128], f16, kind="ExternalInput")
    B1 = nc.dram_tensor("B1", [128, 1], f32, kind="ExternalInput")
    B2 = nc.dram_tensor("B2", [128, 1], f32, kind="ExternalInput")
    B3 = nc.dram_tensor("B3", [128, 1], f32, kind="ExternalInput")
    CB = nc.dram_tensor("CB", [128, 1], f32, kind="ExternalInput")
    LTA = nc.dram_tensor("LTA", [128, _Q], f32, kind="ExternalOutput")
    LTB = nc.dram_tensor("LTB", [128, _Q], f32, kind="ExternalOutput")
    PRD = nc.dram_tensor("PRD", [128, _Q], f32, kind="ExternalOutput")

    with tile.TileContext(nc) as tc:
        with (
            tc.tile_pool(name="w", bufs=1) as wp,
            tc.tile_pool(name="xin", bufs=3) as xp,
            tc.tile_pool(name="ain", bufs=3) as app,
            tc.tile_pool(name="hid", bufs=3) as hp,
            tc.tile_pool(name="outs", bufs=3) as op,
            tc.tile_pool(name="ps", bufs=8, space="PSUM") as psp,
        ):
            w1s = wp.tile([128, 128], f16, tag="w1")   # two stacked W1 blockdiags
            w2s = wp.tile([128, 128], f16, tag="w2")
            w3s = wp.tile([128, 128], f16, tag="w3")
            wps = wp.tile([128, 64], f16, tag="wp")
            wbs = wp.tile([32, 128], f16, tag="wb")    # 4-quarter blockdiag B@C
            b1s = wp.tile([128, 1], f32, tag="b1")
            b2s = wp.tile([128, 1], f32, tag="b2")
            b3s = wp.tile([128, 1], f32, tag="b3")
            cbs = wp.tile([128, 1], f32, tag="cb")
            for t, d in ((w1s, W1b), (w2s, W2b), (w3s, W3b), (wps, WPb),
                         (wbs, WBb), (b1s, B1), (b2s, B2), (b3s, B3),
                         (cbs, CB)):
                nc.sync.dma_start(out=t[:], in_=d.ap()[:])

            def post(use_act, out_ap, ps_ap, bias, relu):
                # out = [relu](psum + bias), PSUM -> SBUF
                if use_act:
                    nc.scalar.activation(out_ap, ps_ap, Relu if relu else Ident,
                                         bias=bias[:, 0:1], scale=1.0)
                elif relu:
                    nc.vector.tensor_scalar(out_ap, ps_ap, bias[:, 0:1], 0.0,
                                            add, mx)
                else:
                    nc.vector.tensor_scalar(out_ap, ps_ap, bias[:, 0:1], None,
                                            add)

            for s in range(_NSUP):
                c0 = s * _SUPER
                xt = xp.tile([128, _SUPER], f16, tag="x", name=f"x_{s}")
                nc.sync.dma_start(out=xt[:], in_=X.ap()[:, c0:c0 + _SUPER])
                at = app.tile([32, _SUPER], f16, tag="a", name=f"a_{s}")
                nc.sync.dma_start(out=at[:], in_=Aa.ap()[:, c0:c0 + _SUPER])

                h1a = hp.tile([128, _SUPER], f16, tag="h1a", name=f"h1a_{s}")
                h1b = hp.tile([128, _SUPER], f16, tag="h1b", name=f"h1b_{s}")
                h2a = hp.tile([128, _SUPER], f16, tag="h2a", name=f"h2a_{s}")
                h2b = hp.tile([128, _SUPER], f16, tag="h2b", name=f"h2b_{s}")
                lta = op.tile([128, _SUPER], f32, tag="lta", name=f"lta_{s}")
                ltb = op.tile([128, _SUPER], f32, tag="ltb", name=f"ltb_{s}")
                prd = op.tile([128, _SUPER], f32, tag="prd", name=f"prd_{s}")

                for j in range(_NJ):
                    sl = slice(j * _NT, (j + 1) * _NT)
                    p1a = psp.tile([128, _NT], f32, tag="ps", name=f"p1a_{s}{j}")
                    p1b = psp.tile([128, _NT], f32, tag="ps", name=f"p1b_{s}{j}")
                    nc.tensor.matmul(p1a[:], w1s[0:64, :], xt[0:64, sl],
                                     start=True, stop=True)
                    nc.tensor.matmul(p1b[:], w1s[64:128, :], xt[64:128, sl],
                                     start=True, stop=True)
                    post(True, h1a[:, sl], p1a[:], b1s, True)
                    post(False, h1b[:, sl], p1b[:], b1s, True)

                    p2a = psp.tile([128, _NT], f32, tag="ps", name=f"p2a_{s}{j}")
                    p2b = psp.tile([128, _NT], f32, tag="ps", name=f"p2b_{s}{j}")
                    nc.tensor.matmul(p2a[:], w2s[:], h1a[:, sl],
                                     start=True, stop=True)
                    nc.tensor.matmul(p2b[:], w2s[:], h1b[:, sl],
                                     start=True, stop=True)
                    post(False, h2a[:, sl], p2a[:], b2s, True)
                    post(True, h2b[:, sl], p2b[:], b2s, True)

                    p3a = psp.tile([128, _NT], f32, tag="ps", name=f"p3a_{s}{j}")
                    p3b = psp.tile([128, _NT], f32, tag="ps", name=f"p3b_{s}{j}")
                    nc.tensor.matmul(p3a[:], w3s[:], h2a[:, sl],
                                     start=True, stop=True)
                    nc.tensor.matmul(p3b[:], w3s[:], h2b[:, sl],
                                     start=True, stop=True)
                    post(True, lta[:, sl], p3a[:], b3s, False)
                    post(False, ltb[:, sl], p3b[:], b3s, False)

                    p4 = psp.tile([128, _NT], f32, tag="ps", name=f"p4_{s}{j}")
                    nc.tensor.matmul(p4[:], wbs[:], at[:, sl],
                                     start=True, stop=False)
                    nc.tensor.matmul(p4[0:64, :], wps[:], h2a[:, sl],
                                     start=False, stop=True)
                    nc.tensor.matmul(p4[64:128, :], wps[:], h2b[:, sl],
                                     start=False, stop=True)
                    post(j % 2 == 0, prd[:, sl], p4[:], cbs, False)

                nc.sync.dma_start(out=LTA.ap()[:, c0:c0 + _SUPER], in_=lta[:])
                nc.sync.dma_start(out=LTB.ap()[:, c0:c0 + _SUPER], in_=ltb[:])
                nc.sync.dma_start(out=PRD.ap()[:, c0:c0 + _SUPER], in_=prd[:])

    nc.compile()
    _prog_cache["nc"] = nc
    return nc


def _host_prep(states, actions, W1, b1, W2, b2, W3, b3, A, B, C,
               state_shift, state_scale, action_shift, action_scale):
    states = np.asarray(states, np.float32)
    actions = np.asarray(actions, np.float32)
    W1 = np.asarray(W1, np.float32); b1 = np.asarray(b1, np.float32)
    W2 = np.asarray(W2, np.float32); b2 = np.asarray(b2, np.float32)
    W3 = np.asarray(W3, np.float32); b3 = np.asarray(b3, np.float32)
    A = np.asarray(A, np.float32); B = np.asarray(B, np.float32)
    C = np.asarray(C, np.float32)
    state_shift = np.asarray(state_shift, np.float32)
    state_scale = np.asarray(state_scale, np.float32)
    action_shift = np.asarray(action_shift, np.float32)
    action_scale = np.asarray(action_scale, np.float32)

    # fold normalization into layer 1
    W1p = W1 / state_scale[:, None]
    b1p = b1 - (state_shift / state_scale) @ W1
    # prediction head: pred = h2 @ (W3@A@C) + act @ (B@C/ascale) + cbias
    AC = A @ C
    WP = W3 @ AC                                  # [64, 32]
    BCp = (B @ C) / action_scale[:, None]         # [8, 32]
    cb = b3 @ AC - (action_shift / action_scale) @ (B @ C)

    w1blk = np.zeros((64, 128), np.float16)
    w1blk[:32, :64] = W1p
    w1blk[32:, 64:] = W1p
    W1b = np.concatenate([w1blk, w1blk], axis=0)   # [128, 128] two row copies

    def blk2(w):
        out = np.zeros((128, 2 * w.shape[1]), np.float16)
        out[:64, :w.shape[1]] = w
        out[64:, w.shape[1]:] = w
        return out

    WBb = np.zeros((32, 128), np.float16)
    for q in range(4):
        WBb[8 * q:8 * q + 8, 32 * q:32 * q + 32] = BCp

    weights = {
        "W1b": W1b,
        "W2b": blk2(W2),
        "W3b": blk2(W3),
        "WPb": blk2(WP),
        "WBb": WBb,
        "B1": np.concatenate([b1p, b1p]).reshape(128, 1).astype(np.float32),
        "B2": np.concatenate([b2, b2]).reshape(128, 1).astype(np.float32),
        "B3": np.concatenate([b3, b3]).reshape(128, 1).astype(np.float32),
        "CB": np.tile(cb, 4).reshape(128, 1).astype(np.float32),
    }

    in_maps = []
    for c in range(_N_CORES):
        st = states[c * _BPC:(c + 1) * _BPC].reshape(_TOK, _SD)
        Xc = np.ascontiguousarray(
            st.reshape(4, _Q, _SD).transpose(0, 2, 1).reshape(128, _Q)
        ).astype(np.float16)
        ac = np.zeros((_BPC, _S, _AD), np.float32)
        ac[:, :_S - 1] = actions[c * _BPC:(c + 1) * _BPC]
        Ac = np.ascontiguousarray(
            ac.reshape(4, _Q, _AD).transpose(0, 2, 1).reshape(32, _Q)
        ).astype(np.float16)
        m = {"X": Xc, "Aa": Ac}
        m.update(weights)
        in_maps.append(m)
    return in_maps


def _assemble(results):
    latent = np.empty((_B, _S, _LD), np.float32)
    preds = np.empty((_B, _S - 1, _SD), np.float32)
    for c in range(_N_CORES):
        lat = np.empty((_TOK, _LD), np.float32)
        r = results[c]
        lat[0 * _Q:1 * _Q] = r["LTA"][0:64].T
        lat[1 * _Q:2 * _Q] = r["LTA"][64:128].T
        lat[2 * _Q:3 * _Q] = r["LTB"][0:64].T
        lat[3 * _Q:4 * _Q] = r["LTB"][64:128].T
        prd = np.empty((_TOK, _SD), np.float32)
        for q in range(4):
            prd[q * _Q:(q + 1) * _Q] = r["PRD"][32 * q:32 * q + 32].T
        latent[c * _BPC:(c + 1) * _BPC] = lat.reshape(_BPC, _S, _LD)
        preds[c * _BPC:(c + 1) * _BPC] = \
            prd.reshape(_BPC, _S, _SD)[:, :_S - 1]
    return preds, latent


def kernel(states, actions, W1, b1, W2, b2, W3, b3, A, B, C,
           state_shift, state_scale, action_shift, action_scale):
    from concourse.bass_utils import run_bass_kernel_spmd

    in_maps = _host_prep(states, actions, W1, b1, W2, b2, W3, b3, A, B, C,
                         state_shift, state_scale, action_shift, action_scale)
    nc = _build_program()
    res = run_bass_kernel_spmd(nc, in_maps, list(range(_N_CORES)))
    return _assemble(res.results)


# revision 4
# speedup vs baseline: 1.1692x; 1.1692x over previous
# Koopman operator model on 8 NeuronCores — pure data parallel over batch.
#
# Device kernel works in a feature-major, token-column layout: every tensor is
# [features, tokens-columns]. The 32768 tokens per core are split into 4
# quarters of 8192; column c carries tokens {q*8192+c}. States are packed
# [128, 8192] (4 quarters x 32 features) so every DMA uses all 128 SBUF
# partitions. The MLP runs as two "pair" streams (quarters 0+1 on partitions
# 0-63 / 64-127 of the weights via block-diagonal packing, quarters 2+3 via
# tile_position row/col offsets), and the prediction head accumulates three
# matmuls (actions quad-block-diag + two pair h2@W3AC) into one PSUM tile.
#
# Math folding (host, exact): normalization into W1/b1; the head
#   pred = lat@A@C + norm_act@B@C  ==  h2@(W3@A@C) + act@(B@C/ascale) + cbias
# so the kernel is 4 matmul passes over tokens + the small action pass.
# All matmul operands are fp16 (1 cycle/row on PE vs 4 for fp32); accumulation
# is fp32 in PSUM; bias+ReLU epilogues run on ACT/DVE; outputs stored fp32.
# Measured end-to-end scale-relative error ~5e-4.

import numpy as np

_N_CORES = 8
_B, _S = 256, 1024
_SD, _AD, _LD = 32, 8, 64
_BPC = _B // _N_CORES            # batches per core
_TOK = _BPC * _S                 # tokens per core (32768)
_Q = _TOK // 4                   # columns per core (4 stacked quarters)
_NT = 512                        # matmul moving-dim tile (PSUM bank limit)
_SUPER = 2048                    # columns per DMA super-tile
_GRP = 1024                      # columns per compute group (PSUM-sized)
_NSUP = _Q // _SUPER
_NG = _SUPER // _GRP
_NJ = _GRP // _NT

_prog_cache = {}


def _build_program():
    if "nc" in _prog_cache:
        return _prog_cache["nc"]

    import concourse.mybir as mybir
    import concourse.tile as tile
    from concourse import bacc

    f16, f32 = mybir.dt.float16, mybir.dt.float32
    Relu = mybir.ActivationFunctionType.Relu
    Ident = mybir.ActivationFunctionType.Identity
    add, mx = mybir.AluOpType.add, mybir.AluOpType.max

    nc = bacc.Bacc("TRN2", target_bir_lowering=False, debug=False,
                   num_devices=_N_CORES)

    X = nc.dram_tensor("X", [128, _Q], f16, kind="ExternalInput")
    Aa = nc.dram_tensor("Aa", [32, _Q], f16, kind="ExternalInput")
    W1b = nc.dram_tensor("W1b", [128, 128], f16, kind="ExternalInput")
    W2b = nc.dram_tensor("W2b", [128, 128], f16, kind="ExternalInput")
    W3b = nc.dram_tensor("W3b", [128, 128], f16, kind="ExternalInput")
    WPb = nc.dram_tensor("WPb", [128, 64], f16, kind="ExternalInput")
    WBb = nc.dram_tensor("WBb", [32, 128], f16, kind="ExternalInput")
    B1 = nc.dram_tensor("B1", [128, 1], f32, kind="ExternalInput")
    B2 = nc.dram_tensor("B2", [128, 1], f32, kind="ExternalInput")
    B3 = nc.dram_tensor("B3", [128, 1], f32, kind="ExternalInput")
    CB = nc.dram_tensor("CB", [128, 1], f32, kind="ExternalInput")
    LTA = nc.dram_tensor("LTA", [128, _Q], f32, kind="ExternalOutput")
    LTB = nc.dram_tensor("LTB", [128, _Q], f32, kind="ExternalOutput")
    PRD = nc.dram_tensor("PRD", [128, _Q], f32, kind="ExternalOutput")

    with tile.TileContext(nc) as tc:
        with (
            tc.tile_pool(name="w", bufs=1) as wp,
            tc.tile_pool(name="xin", bufs=3) as xp,
            tc.tile_pool(name="ain", bufs=3) as app,
            tc.tile_pool(name="hid", bufs=3) as hp,
            tc.tile_pool(name="outs", bufs=3) as op,
            tc.tile_pool(name="ps", bufs=8, space="PSUM") as psp,
        ):
            w1s = wp.tile([128, 128], f16, tag="w1")   # two stacked W1 blockdiags
            w2s = wp.tile([128, 128], f16, tag="w2")
            w3s = wp.tile([128, 128], f16, tag="w3")
            wps = wp.tile([128, 64], f16, tag="wp")
            wbs = wp.tile([32, 128], f16, tag="wb")    # 4-quarter blockdiag B@C
            b1s = wp.tile([128, 1], f32, tag="b1")
            b2s = wp.tile([128, 1], f32, tag="b2")
            b3s = wp.tile([128, 1], f32, tag="b3")
            cbs = wp.tile([128, 1], f32, tag="cb")
            for t, d in ((w1s, W1b), (w2s, W2b), (w3s, W3b), (wps, WPb),
                         (wbs, WBb), (b1s, B1), (b2s, B2), (b3s, B3),
                         (cbs, CB)):
                nc.sync.dma_start(out=t[:], in_=d.ap()[:])

            def post(use_act, out_ap, ps_ap, bias, relu):
                # out = [relu](psum + bias), PSUM -> SBUF
                if use_act:
                    nc.scalar.activation(out_ap, ps_ap, Relu if relu else Ident,
                                         bias=bias[:, 0:1], scale=1.0)
                elif relu:
                    nc.vector.tensor_scalar(out_ap, ps_ap, bias[:, 0:1], 0.0,
                                            add, mx)
                else:
                    nc.vector.tensor_scalar(out_ap, ps_ap, bias[:, 0:1], None,
                                            add)

            for s in range(_NSUP):
                c0 = s * _SUPER
                xt = xp.tile([128, _SUPER], f16, tag="x", name=f"x_{s}")
                nc.sync.dma_start(out=xt[:], in_=X.ap()[:, c0:c0 + _SUPER])
                at = app.tile([32, _SUPER], f16, tag="a", name=f"a_{s}")
                nc.sync.dma_start(out=at[:], in_=Aa.ap()[:, c0:c0 + _SUPER])

                h1a = hp.tile([128, _SUPER], f16, tag="h1a", name=f"h1a_{s}")
                h1b = hp.tile([128, _SUPER], f16, tag="h1b", name=f"h1b_{s}")
                h2a = hp.tile([128, _SUPER], f16, tag="h2a", name=f"h2a_{s}")
                h2b = hp.tile([128, _SUPER], f16, tag="h2b", name=f"h2b_{s}")
                lta = op.tile([128, _SUPER], f32, tag="lta", name=f"lta_{s}")
                ltb = op.tile([128, _SUPER], f32, tag="ltb", name=f"ltb_{s}")
                prd = op.tile([128, _SUPER], f32, tag="prd", name=f"prd_{s}")

                # layer-major emission per compute group: PE streams a whole
                # layer's matmuls back-to-back while ACT/DVE run the previous
                # layer's epilogues, so PE never stalls on a single chain.
                for g in range(_NG):
                    sls = [slice(g * _GRP + j * _NT, g * _GRP + (j + 1) * _NT)
                           for j in range(_NJ)]

                    p1 = [psp.tile([128, _NT], f32, tag="ps",
                                   name=f"p1{h}_{s}{g}{j}")
                          for j in range(_NJ) for h in ("a", "b")]
                    for j in range(_NJ):
                        nc.tensor.matmul(p1[2 * j][:], w1s[0:64, :],
                                         xt[0:64, sls[j]],
                                         start=True, stop=True)
                        nc.tensor.matmul(p1[2 * j + 1][:], w1s[64:128, :],
                                         xt[64:128, sls[j]],
                                         start=True, stop=True)
                    for j in range(_NJ):
                        post(True, h1a[:, sls[j]], p1[2 * j][:], b1s, True)
                        post(False, h1b[:, sls[j]], p1[2 * j + 1][:], b1s, True)

                    p2 = [psp.tile([128, _NT], f32, tag="ps",
                                   name=f"p2{h}_{s}{g}{j}")
                          for j in range(_NJ) for h in ("a", "b")]
                    for j in range(_NJ):
                        nc.tensor.matmul(p2[2 * j][:], w2s[:], h1a[:, sls[j]],
                                         start=True, stop=True)
                        nc.tensor.matmul(p2[2 * j + 1][:], w2s[:],
                                         h1b[:, sls[j]],
                                         start=True, stop=True)
                    for j in range(_NJ):
                        post(False, h2a[:, sls[j]], p2[2 * j][:], b2s, True)
                        post(True, h2b[:, sls[j]], p2[2 * j + 1][:], b2s, True)

                    p3 = [psp.tile([128, _NT], f32, tag="ps",
                                   name=f"p3{h}_{s}{g}{j}")
                          for j in range(_NJ) for h in ("a", "b")]
                    for j in range(_NJ):
                        nc.tensor.matmul(p3[2 * j][:], w3s[:], h2a[:, sls[j]],
                                         start=True, stop=True)
                        nc.tensor.matmul(p3[2 * j + 1][:], w3s[:],
                                         h2b[:, sls[j]],
                                         start=True, stop=True)
                    for j in range(_NJ):
                        post(True, lta[:, sls[j]], p3[2 * j][:], b3s, False)
                        post(False, ltb[:, sls[j]], p3[2 * j + 1][:], b3s, False)

                    p4 = [psp.tile([128, _NT], f32, tag="ps",
                                   name=f"p4_{s}{g}{j}")
                          for j in range(_NJ)]
                    for j in range(_NJ):
                        nc.tensor.matmul(p4[j][:], wbs[:], at[:, sls[j]],
                                         start=True, stop=False)
                        nc.tensor.matmul(p4[j][0:64, :], wps[:],
                                         h2a[:, sls[j]],
                                         start=False, stop=True)
                        nc.tensor.matmul(p4[j][64:128, :], wps[:],
                                         h2b[:, sls[j]],
                                         start=False, stop=True)
                    for j in range(_NJ):
                        post(j % 2 == 0, prd[:, sls[j]], p4[j][:], cbs, False)

                nc.sync.dma_start(out=LTA.ap()[:, c0:c0 + _SUPER], in_=lta[:])
                nc.sync.dma_start(out=LTB.ap()[:, c0:c0 + _SUPER], in_=ltb[:])
                nc.sync.dma_start(out=PRD.ap()[:, c0:c0 + _SUPER], in_=prd[:])

    nc.compile()
    _prog_cache["nc"] = nc
    return nc


def _host_prep(states, actions, W1, b1, W2, b2, W3, b3, A, B, C,
               state_shift, state_scale, action_shift, action_scale):
    states = np.asarray(states, np.float32)
    actions = np.asarray(actions, np.float32)
    W1 = np.asarray(W1, np.float32); b1 = np.asarray(b1, np.float32)
    W2 = np.asarray(W2, np.float32); b2 = np.asarray(b2, np.float32)
    W3 = np.asarray(W3, np.float32); b3 = np.asarray(b3, np.float32)
    A = np.asarray(A, np.float32); B = np.asarray(B, np.float32)
    C = np.asarray(C, np.float32)
    state_shift = np.asarray(state_shift, np.float32)
    state_scale = np.asarray(state_scale, np.float32)
    action_shift = np.asarray(action_shift, np.float32)
    action_scale = np.asarray(action_scale, np.float32)

    # fold normalization into layer 1
    W1p = W1 / state_scale[:, None]
    b1p = b1 - (state_shift / state_scale) @ W1
    # prediction head: pred = h2 @ (W3@A@C) + act @ (B@C/ascale) + cbias
    AC = A @ C
    WP = W3 @ AC                                  # [64, 32]
    BCp = (B @ C) / action_scale[:, None]         # [8, 32]
    cb = b3 @ AC - (action_shift / action_scale) @ (B @ C)

    w1blk = np.zeros((64, 128), np.float16)
    w1blk[:32, :64] = W1p
    w1blk[32:, 64:] = W1p
    W1b = np.concatenate([w1blk, w1blk], axis=0)   # [128, 128] two row copies

    def blk2(w):
        out = np.zeros((128, 2 * w.shape[1]), np.float16)
        out[:64, :w.shape[1]] = w
        out[64:, w.shape[1]:] = w
        return out

    WBb = np.zeros((32, 128), np.float16)
    for q in range(4):
        WBb[8 * q:8 * q + 8, 32 * q:32 * q + 32] = BCp

    weights = {
        "W1b": W1b,
        "W2b": blk2(W2),
        "W3b": blk2(W3),
        "WPb": blk2(WP),
        "WBb": WBb,
        "B1": np.concatenate([b1p, b1p]).reshape(128, 1).astype(np.float32),
        "B2": np.concatenate([b2, b2]).reshape(128, 1).astype(np.float32),
        "B3": np.concatenate([b3, b3]).reshape(128, 1).astype(np.float32),
        "CB": np.tile(cb, 4).reshape(128, 1).astype(np.float32),
    }

    in_maps = []
    for c in range(_N_CORES):
        st = states[c * _BPC:(c + 1) * _BPC].reshape(_TOK, _SD)
        Xc = np.ascontiguousarray(
            st.reshape(4, _Q, _SD).transpose(0, 2, 1).reshape(128, _Q)
        ).astype(np.float16)
        ac = np.zeros((_BPC, _S, _AD), np.float32)
        ac[:, :_S - 1] = actions[c * _BPC:(c + 1) * _BPC]
        Ac = np.ascontiguousarray(
            ac.reshape(4, _Q, _AD).transpose(0, 2, 1).reshape(32, _Q)
        ).astype(np.float16)
        m = {"X": Xc, "Aa": Ac}
        m.update(weights)
        in_maps.append(m)
    return in_maps


def _assemble(results):
    latent = np.empty((_B, _S, _LD), np.float32)
    preds = np.empty((_B, _S - 1, _SD), np.float32)
    for c in range(_N_CORES):
        lat = np.empty((_TOK, _LD), np.float32)
        r = results[c]
        lat[0 * _Q:1 * _Q] = r["LTA"][0:64].T
        lat[1 * _Q:2 * _Q] = r["LTA"][64:128].T
        lat[2 * _Q:3 * _Q] = r["LTB"][0:64].T
        lat[3 * _Q:4 * _Q] = r["LTB"][64:128].T
        prd = np.empty((_TOK, _SD), np.float32)
        for q in range(4):
            prd[q * _Q:(q + 1) * _Q] = r["PRD"][32 * q:32 * q + 32].T
        latent[c * _BPC:(c + 1) * _BPC] = lat.reshape(_BPC, _S, _LD)
        preds[c * _BPC:(c + 1) * _BPC] = \
            prd.reshape(_BPC, _S, _SD)[:, :_S - 1]
    return preds, latent


def kernel(states, actions, W1, b1, W2, b2, W3, b3, A, B, C,
           state_shift, state_scale, action_shift, action_scale):
    from concourse.bass_utils import run_bass_kernel_spmd

    in_maps = _host_prep(states, actions, W1, b1, W2, b2, W3, b3, A, B, C,
                         state_shift, state_scale, action_shift, action_scale)
    nc = _build_program()
    res = run_bass_kernel_spmd(nc, in_maps, list(range(_N_CORES)))
    return _assemble(res.results)


# revision 5
# speedup vs baseline: 1.2341x; 1.0555x over previous
# Koopman operator model on 8 NeuronCores — pure data parallel over batch.
#
# Device kernel works in a feature-major, token-column layout: every tensor is
# [features, tokens-columns]. The 32768 tokens per core are split into 4
# quarters of 8192; column c carries tokens {q*8192+c}. States are packed
# [128, 8192] (4 quarters x 32 features) so every DMA uses all 128 SBUF
# partitions. The MLP runs as two "pair" streams (quarters 0+1 on weight
# partitions 0-63, quarters 2+3 on 64-127 via block-diagonal packing and
# tile_position row offsets), and the prediction head accumulates three
# matmuls (actions quad-block-diag + two pair h2@W3AC) into one PSUM tile.
#
# Math folding (host, exact): normalization into W1/b1; the head
#   pred = lat@A@C + norm_act@B@C  ==  h2@(W3@A@C) + act@(B@C/ascale) + cbias
# so the kernel is 4 matmul passes over tokens + the small action pass.
# All matmul operands are fp16 (1 cycle/row on PE vs 4 for fp32); accumulation
# is fp32 in PSUM; bias+ReLU epilogues run on ACT/DVE; outputs are stored fp16
# and upcast on the host. Measured end-to-end scale-relative error ~6e-4.

import numpy as np

_N_CORES = 8
_B, _S = 256, 1024
_SD, _AD, _LD = 32, 8, 64
_BPC = _B // _N_CORES            # batches per core
_TOK = _BPC * _S                 # tokens per core (32768)
_Q = _TOK // 4                   # columns per core (4 stacked quarters)
_NT = 512                        # matmul moving-dim tile (PSUM bank limit)
_GRP = 1024                      # columns per compute group (2 PSUM banks)
_SUPER = 2048                    # columns per input DMA super-tile
_NSUP = _Q // _SUPER
_NG = _SUPER // _GRP
_NJ = _GRP // _NT

_prog_cache = {}


def _build_program():
    if "nc" in _prog_cache:
        return _prog_cache["nc"]

    import concourse.mybir as mybir
    import concourse.tile as tile
    from concourse import bacc

    f16, f32 = mybir.dt.float16, mybir.dt.float32
    Relu = mybir.ActivationFunctionType.Relu
    Ident = mybir.ActivationFunctionType.Identity
    add, mx = mybir.AluOpType.add, mybir.AluOpType.max

    nc = bacc.Bacc("TRN2", target_bir_lowering=False, debug=False,
                   num_devices=_N_CORES)

    X = nc.dram_tensor("X", [128, _Q], f16, kind="ExternalInput")
    Aa = nc.dram_tensor("Aa", [32, _Q], f16, kind="ExternalInput")
    # all fp16 weights packed in one tensor: W1b|W2b|W3b|WPb|WBb
    WALL = nc.dram_tensor("WALL", [128, 576], f16, kind="ExternalInput")
    BALL = nc.dram_tensor("BALL", [128, 4], f32, kind="ExternalInput")
    LTA = nc.dram_tensor("LTA", [128, _Q], f16, kind="ExternalOutput")
    LTB = nc.dram_tensor("LTB", [128, _Q], f16, kind="ExternalOutput")
    PRD = nc.dram_tensor("PRD", [128, _Q], f16, kind="ExternalOutput")

    with tile.TileContext(nc) as tc:
        with (
            tc.tile_pool(name="w", bufs=1) as wp,
            tc.tile_pool(name="xin", bufs=3) as xp,
            tc.tile_pool(name="ain", bufs=3) as app,
            tc.tile_pool(name="hid", bufs=3) as hp,
            tc.tile_pool(name="outs", bufs=3) as op,
            tc.tile_pool(name="ps", bufs=4, space="PSUM") as psp,
        ):
            wall = wp.tile([128, 576], f16, tag="wall")
            ball = wp.tile([128, 4], f32, tag="ball")
            nc.sync.dma_start(out=wall[:], in_=WALL.ap()[:])
            nc.sync.dma_start(out=ball[:], in_=BALL.ap()[:])
            w1s = wall[:, 0:128]       # two stacked W1 blockdiags
            w2s = wall[:, 128:256]
            w3s = wall[:, 256:384]
            wps = wall[:, 384:448]
            wbs = wall[0:32, 448:576]  # 4-quarter blockdiag B@C/ascale
            b1s, b2s, b3s, cbs = (ball[:, i:i + 1] for i in range(4))

            def post(use_act, out_ap, ps_ap, bias, relu):
                # out = [relu](psum + bias), PSUM -> SBUF
                if use_act:
                    nc.scalar.activation(out_ap, ps_ap, Relu if relu else Ident,
                                         bias=bias, scale=1.0)
                elif relu:
                    nc.vector.tensor_scalar(out_ap, ps_ap, bias, 0.0, add, mx)
                else:
                    nc.vector.tensor_scalar(out_ap, ps_ap, bias, None, add)

            for s in range(_NSUP):
                c0 = s * _SUPER
                xt = xp.tile([128, _SUPER], f16, tag="x", name=f"x_{s}")
                nc.sync.dma_start(out=xt[:], in_=X.ap()[:, c0:c0 + _SUPER])
                at = app.tile([32, _SUPER], f16, tag="a", name=f"a_{s}")
                nc.sync.dma_start(out=at[:], in_=Aa.ap()[:, c0:c0 + _SUPER])

                h1a = hp.tile([128, _SUPER], f16, tag="h1a", name=f"h1a_{s}")
                h1b = hp.tile([128, _SUPER], f16, tag="h1b", name=f"h1b_{s}")
                h2a = hp.tile([128, _SUPER], f16, tag="h2a", name=f"h2a_{s}")
                h2b = hp.tile([128, _SUPER], f16, tag="h2b", name=f"h2b_{s}")
                lta = op.tile([128, _SUPER], f16, tag="lta", name=f"lta_{s}")
                ltb = op.tile([128, _SUPER], f16, tag="ltb", name=f"ltb_{s}")
                prd = op.tile([128, _SUPER], f16, tag="prd", name=f"prd_{s}")

                # layer-major emission per compute group: PE streams a whole
                # layer's matmuls back-to-back while ACT/DVE run the previous
                # layer's epilogues. PSUM tiles are [128, 1024] (2 banks, one
                # per 512-column matmul) so each epilogue covers 1024 columns.
                for g in range(_NG):
                    gsl = slice(g * _GRP, (g + 1) * _GRP)
                    sls = [slice(g * _GRP + j * _NT, g * _GRP + (j + 1) * _NT)
                           for j in range(_NJ)]
                    jsl = [slice(j * _NT, (j + 1) * _NT) for j in range(_NJ)]

                    p1a = psp.tile([128, _GRP], f32, tag="ps", name=f"p1a{s}{g}")
                    p1b = psp.tile([128, _GRP], f32, tag="ps", name=f"p1b{s}{g}")
                    for j in range(_NJ):
                        nc.tensor.matmul(p1a[:, jsl[j]], w1s[0:64, :],
                                         xt[0:64, sls[j]],
                                         start=True, stop=True)
                        nc.tensor.matmul(p1b[:, jsl[j]], w1s[64:128, :],
                                         xt[64:128, sls[j]],
                                         start=True, stop=True)
                    post(True, h1a[:, gsl], p1a[:], b1s, True)
                    post(False, h1b[:, gsl], p1b[:], b1s, True)

                    p2a = psp.tile([128, _GRP], f32, tag="ps", name=f"p2a{s}{g}")
                    p2b = psp.tile([128, _GRP], f32, tag="ps", name=f"p2b{s}{g}")
                    for j in range(_NJ):
                        nc.tensor.matmul(p2a[:, jsl[j]], w2s, h1a[:, sls[j]],
                                         start=True, stop=True)
                        nc.tensor.matmul(p2b[:, jsl[j]], w2s, h1b[:, sls[j]],
                                         start=True, stop=True)
                    post(False, h2a[:, gsl], p2a[:], b2s, True)
                    post(True, h2b[:, gsl], p2b[:], b2s, True)

                    p3a = psp.tile([128, _GRP], f32, tag="ps", name=f"p3a{s}{g}")
                    p3b = psp.tile([128, _GRP], f32, tag="ps", name=f"p3b{s}{g}")
                    for j in range(_NJ):
                        nc.tensor.matmul(p3a[:, jsl[j]], w3s, h2a[:, sls[j]],
                                         start=True, stop=True)
                        nc.tensor.matmul(p3b[:, jsl[j]], w3s, h2b[:, sls[j]],
                                         start=True, stop=True)
                    post(True, lta[:, gsl], p3a[:], b3s, False)
                    post(False, ltb[:, gsl], p3b[:], b3s, False)

                    p4 = psp.tile([128, _GRP], f32, tag="ps", name=f"p4{s}{g}")
                    for j in range(_NJ):
                        nc.tensor.matmul(p4[:, jsl[j]], wbs, at[:, sls[j]],
                                         start=True, stop=False)
                        nc.tensor.matmul(p4[0:64, jsl[j]], wps,
                                         h2a[:, sls[j]],
                                         start=False, stop=True)
                        nc.tensor.matmul(p4[64:128, jsl[j]], wps,
                                         h2b[:, sls[j]],
                                         start=False, stop=True)
                    post(g % 2 == 0, prd[:, gsl], p4[:], cbs, False)

                    nc.sync.dma_start(out=LTA.ap()[:, c0 + g * _GRP:
                                                   c0 + (g + 1) * _GRP],
                                      in_=lta[:, gsl])
                    nc.sync.dma_start(out=LTB.ap()[:, c0 + g * _GRP:
                                                   c0 + (g + 1) * _GRP],
                                      in_=ltb[:, gsl])
                    nc.sync.dma_start(out=PRD.ap()[:, c0 + g * _GRP:
                                                   c0 + (g + 1) * _GRP],
                                      in_=prd[:, gsl])

    nc.compile()
    _prog_cache["nc"] = nc
    return nc


def _host_prep(states, actions, W1, b1, W2, b2, W3, b3, A, B, C,
               state_shift, state_scale, action_shift, action_scale):
    states = np.asarray(states, np.float32)
    actions = np.asarray(actions, np.float32)
    W1 = np.asarray(W1, np.float32); b1 = np.asarray(b1, np.float32)
    W2 = np.asarray(W2, np.float32); b2 = np.asarray(b2, np.float32)
    W3 = np.asarray(W3, np.float32); b3 = np.asarray(b3, np.float32)
    A = np.asarray(A, np.float32); B = np.asarray(B, np.float32)
    C = np.asarray(C, np.float32)
    state_shift = np.asarray(state_shift, np.float32)
    state_scale = np.asarray(state_scale, np.float32)
    action_shift = np.asarray(action_shift, np.float32)
    action_scale = np.asarray(action_scale, np.float32)

    # fold normalization into layer 1
    W1p = W1 / state_scale[:, None]
    b1p = b1 - (state_shift / state_scale) @ W1
    # prediction head: pred = h2 @ (W3@A@C) + act @ (B@C/ascale) + cbias
    AC = A @ C
    WP = W3 @ AC                                  # [64, 32]
    BCp = (B @ C) / action_scale[:, None]         # [8, 32]
    cb = b3 @ AC - (action_shift / action_scale) @ (B @ C)

    w1blk = np.zeros((64, 128), np.float16)
    w1blk[:32, :64] = W1p
    w1blk[32:, 64:] = W1p

    def blk2(w):
        out = np.zeros((128, 2 * w.shape[1]), np.float16)
        out[:64, :w.shape[1]] = w
        out[64:, w.shape[1]:] = w
        return out

    wall = np.zeros((128, 576), np.float16)
    wall[:, 0:128] = np.concatenate([w1blk, w1blk], axis=0)
    wall[:, 128:256] = blk2(W2)
    wall[:, 256:384] = blk2(W3)
    wall[:, 384:448] = blk2(WP)
    for q in range(4):
        wall[8 * q:8 * q + 8, 448 + 32 * q:448 + 32 * q + 32] = BCp

    ball = np.zeros((128, 4), np.float32)
    ball[:, 0] = np.concatenate([b1p, b1p])
    ball[:, 1] = np.concatenate([b2, b2])
    ball[:, 2] = np.concatenate([b3, b3])
    ball[:, 3] = np.tile(cb, 4)

    weights = {"WALL": wall, "BALL": ball}

    in_maps = []
    for c in range(_N_CORES):
        st = states[c * _BPC:(c + 1) * _BPC].reshape(_TOK, _SD)
        Xc = np.ascontiguousarray(
            st.reshape(4, _Q, _SD).transpose(0, 2, 1).reshape(128, _Q)
        ).astype(np.float16)
        ac = np.zeros((_BPC, _S, _AD), np.float32)
        ac[:, :_S - 1] = actions[c * _BPC:(c + 1) * _BPC]
        Ac = np.ascontiguousarray(
            ac.reshape(4, _Q, _AD).transpose(0, 2, 1).reshape(32, _Q)
        ).astype(np.float16)
        m = {"X": Xc, "Aa": Ac}
        m.update(weights)
        in_maps.append(m)
    return in_maps


def _assemble(results):
    latent = np.empty((_B, _S, _LD), np.float32)
    preds = np.empty((_B, _S - 1, _SD), np.float32)
    for c in range(_N_CORES):
        lat = np.empty((_TOK, _LD), np.float32)
        r = results[c]
        lat[0 * _Q:1 * _Q] = r["LTA"][0:64].T
        lat[1 * _Q:2 * _Q] = r["LTA"][64:128].T
        lat[2 * _Q:3 * _Q] = r["LTB"][0:64].T
        lat[3 * _Q:4 * _Q] = r["LTB"][64:128].T
        prd = np.empty((_TOK, _SD), np.float32)
        for q in range(4):
            prd[q * _Q:(q + 1) * _Q] = r["PRD"][32 * q:32 * q + 32].T
        latent[c * _BPC:(c + 1) * _BPC] = lat.reshape(_BPC, _S, _LD)
        preds[c * _BPC:(c + 1) * _BPC] = \
            prd.reshape(_BPC, _S, _SD)[:, :_S - 1]
    return preds, latent


def kernel(states, actions, W1, b1, W2, b2, W3, b3, A, B, C,
           state_shift, state_scale, action_shift, action_scale):
    from concourse.bass_utils import run_bass_kernel_spmd

    in_maps = _host_prep(states, actions, W1, b1, W2, b2, W3, b3, A, B, C,
                         state_shift, state_scale, action_shift, action_scale)
    nc = _build_program()
    res = run_bass_kernel_spmd(nc, in_maps, list(range(_N_CORES)))
    return _assemble(res.results)
